# revision 2
# baseline (speedup 1.0000x reference)
"""Trainium2 kernel for nn_BetweennessRoPE.

Mathematical background
-----------------------
The reference computes a "betweenness"-adjusted interpolated RoPE:

    adjust      = gate * (betweenness - 0.5) * 0.1
    adj_pos     = clip(pos + adjust, 0, 2047)
    cos_i/sin_i = lerp of the cos/sin tables at floor/ceil(adj_pos)
    out         = rotate(x, cos_i, sin_i)

By the triangle inequality path >= direct, so score in [0, 1] and
betweenness in [0, 1/(L-2)].  Hence

    adjust = gate*0.05*betweenness - gate*0.05  in  (-0.025, -0.0249756]

is always a small negative number: floor/ceil(pos + adjust) = (pos-1, pos)
for every pos >= 1 (and pos 0 clips to exactly 0).  The interpolation
therefore uses *statically known* table rows, with fraction

    frac = 1 + adjust = f0 + eps,   f0 = 1 - 0.05*gate,
    eps  = gate*0.05*betweenness  in  [0, gate*0.05/(L-2)]  (~2.4e-5)

The eps-dependent part of the output is bounded by |eps * x|, two orders
of magnitude below the test gate, so the kernel applies the lerped
rotation at fixed fraction f0 with host-precomputed tables

    Mc[l] = (1-f0)*cos((l-1)*theta) + f0*cos(l*theta)   (l >= 1)
    Ms[l] = (1-f0)*sin((l-1)*theta) + f0*sin(l*theta)
    Mc[0] = 1, Ms[0] = 0                                (pos-0 clips to 0)

and the device kernel is a pure broadcast complex-multiply:

    out_even = x_even*Mc - x_odd*Ms
    out_odd  = x_odd *Mc + x_even*Ms

Memory-bound; data-parallel over batch (core i handles batch i).

Device layout (per core)
------------------------
x is sent de-interleaved in fp16 with per-partition free layout
(h, pr, k, lh) and partitions p = l % 128 (l = lh*128 + p):

    xd[p, h, pr, k, lh]     flat [128, 16384]

so every DMA slice along h is fully contiguous per partition, and the
cos/sin table operands (which do not depend on h) are contiguous
512-element runs broadcast with a stride-0 h (and pr, for cos) axis —
the cheapest possible AP for the DVE's fp16 2x mode.

Per head-group: tP = x*C and tQ = x*(+-S) on DVE (2x fp16), then
out = tP + parity-swap(tQ) on a per-group engine: DVE for the first
and last group (fast ramp / short tail), TensorE identity-matmul
accumulation + ScalarE PSUM->SBUF cast for middle groups, GPSIMD
tensor_tensor for the rest — balancing all engines under the ~23 us
HBM roofline for the 8 MiB of fp16 traffic.
"""

import os
import sys

import numpy as np

for _p in ("/opt/trn_rl_repo",):
    if _p not in sys.path and os.path.isdir(_p):
        sys.path.insert(0, _p)

import concourse.tile as tile  # noqa: E402
from concourse import bacc, mybir  # noqa: E402
from concourse.bass_utils import run_bass_kernel_spmd  # noqa: E402

B, L, H, D = 8, 2048, 16, 64
K = D // 2  # 32
P = 128  # partitions
LH = L // P  # 16 l_hi values
F = K * LH  # 512: per-(h,pr) contiguous run length
NCORES = 8

# Tunables
# h-split per pipeline group (must sum to H=16) and per-group add engine:
# V = DVE tensor_tensor, T = TensorE identity-matmul + ScalarE cast,
# G = GPSIMD tensor_tensor.
HSPLIT = [int(s) for s in os.environ.get("ROPE_HSPLIT", "2,2,2,2,2,2,2,2").split(",")]
ADDMIX = os.environ.get("ROPE_ADDMIX", "VTTTTGGV")
F16 = os.environ.get("ROPE_F16", "1") == "1"  # fp16 pipeline (else fp32)

_cache = {}


def _build(dt_np):
    """Build the Bass program (shared by all 8 cores)."""
    assert sum(HSPLIT) == H and len(ADDMIX) == len(HSPLIT)
    use_te = "T" in ADDMIX
    dt = mybir.dt.float16 if dt_np == np.float16 else mybir.dt.float32
    nc = bacc.Bacc(
        "TRN2",
        target_bir_lowering=False,
        debug=False,
        enable_asserts=False,
        num_devices=NCORES,
    )
    xin = nc.dram_tensor("x", [P, H * 2 * F], dt, kind="ExternalInput")
    ctd = nc.dram_tensor("ct", [P, F], dt, kind="ExternalInput")
    std = nc.dram_tensor("st", [P, 2 * F], dt, kind="ExternalInput")
    if use_te:
        idd = nc.dram_tensor("iden", [P, P], dt, kind="ExternalInput")
    out = nc.dram_tensor("out", [P, H * 2 * F], dt, kind="ExternalOutput")

    from contextlib import ExitStack

    with tile.TileContext(nc) as tc, ExitStack() as ctx:
        tabp = ctx.enter_context(tc.tile_pool(name="tab", bufs=1))
        xp = ctx.enter_context(tc.tile_pool(name="xin", bufs=4))
        op_ = ctx.enter_context(tc.tile_pool(name="out", bufs=4))
        tp = ctx.enter_context(tc.tile_pool(name="tmp", bufs=4))
        olp = ctx.enter_context(tc.tile_pool(name="outl", bufs=1))
        if use_te:
            psp = ctx.enter_context(tc.tile_pool(name="ps", bufs=2, space="PSUM"))

        mult = mybir.AluOpType.mult
        add = mybir.AluOpType.add

        # first x load issues before everything else on the sync ring;
        # tables + identity go down the scalar ring in parallel
        h0 = 0
        nh0 = HSPLIT[0]
        xt0 = xp.tile([P, nh0 * 2 * F], dt, tag="xt")
        nc.sync.dma_start(xt0[:], xin[:, : nh0 * 2 * F])
        ctt = tabp.tile([P, F], dt)
        nc.scalar.dma_start(ctt[:], ctd[:])
        stt = tabp.tile([P, 2 * F], dt)
        nc.scalar.dma_start(stt[:], std[:])
        if use_te:
            idt = tabp.tile([P, P], dt)
            nc.scalar.dma_start(idt[:], idd[:])

        nstore_scalar = (len(HSPLIT) + 1) // 2
        for g, (nh, eng) in enumerate(zip(HSPLIT, ADDMIX)):
            gf = nh * 2 * F
            if g == 0:
                xt = xt0
            else:
                xt = xp.tile([P, gf], dt, tag="xt")
                nc.sync.dma_start(xt[:], xin[:, h0 * 2 * F : h0 * 2 * F + gf])

            # dedicated tile for the last group's output: its combine must
            # never wait on a store-slot recycle
            if g == len(HSPLIT) - 1:
                ot = olp.tile([P, gf], dt)
            else:
                ot = op_.tile([P, gf], dt, tag="ot")

            xv = xt[:].rearrange("p (h pr f) -> p h pr f", h=nh, pr=2)
            ov = ot[:].rearrange("p (h pr f) -> p h pr f", h=nh, pr=2)
            # cos: broadcast over (h, pr); sin: sign-folded per parity,
            # broadcast over h only.  Both have contiguous 512-elem runs.
            C = ctt[:].unsqueeze(1).unsqueeze(1).broadcast_to([P, nh, 2, F])
            S2 = stt[:].rearrange("p (pr f) -> p pr f", pr=2).unsqueeze(1)
            S2 = S2.broadcast_to([P, nh, 2, F])

            tP = tp.tile([P, gf], dt, tag="tP")
            tQ = tp.tile([P, gf], dt, tag="tQ")
            tPv = tP[:].rearrange("p (h pr f) -> p h pr f", h=nh, pr=2)
            tQv = tQ[:].rearrange("p (h pr f) -> p h pr f", h=nh, pr=2)

            # tP = x*C ; tQ = x*(+-S) ; out = tP + parity-swap(tQ):
            #   out_even = E*C + (O*-S) ; out_odd = O*C + (E*+S)
            nc.vector.tensor_tensor(tPv, xv, C, mult)
            nc.vector.tensor_tensor(tQv, xv, S2, mult)

            if eng == "T":
                # the add runs on TensorE as identity-matmul accumulation
                # into PSUM; ScalarE casts PSUM f32 -> SBUF fp16.  A
                # 512-col chunk is one (h, pr) slot; its parity partner
                # is chunk c^1.
                ps = psp.tile([P, gf], mybir.dt.float32, tag="ps")
                for c in range(gf // 512):
                    pch = tP[:, c * 512 : (c + 1) * 512]
                    qch = tQ[:, (c ^ 1) * 512 : ((c ^ 1) + 1) * 512]
                    po = ps[:, c * 512 : (c + 1) * 512]
                    nc.tensor.matmul(po, idt[:], pch, start=True, stop=False)
                    nc.tensor.matmul(po, idt[:], qch, start=False, stop=True)
                nc.scalar.copy(ot[:], ps[:])
            elif eng == "G":
                nc.gpsimd.tensor_tensor(ov, tPv, tQv[:, :, ::-1, :], add)
            else:
                nc.vector.tensor_tensor(ov, tPv, tQv[:, :, ::-1, :], add)

            # stores: first half on the scalar ring, second half on sync
            # (whose loads are all queued by then) to split trigger cost
            if g < nstore_scalar:
                nc.scalar.dma_start(out[:, h0 * 2 * F : h0 * 2 * F + gf], ot[:])
            else:
                nc.sync.dma_start(out[:, h0 * 2 * F : h0 * 2 * F + gf], ot[:])
            h0 += nh

    nc.compile()
    return nc


def _tables(gate_val, dt_np):
    """Host-precomputed lerped cos/sin tables.

    Returns ct [P, F] with ct[p, k*LH+lh] = Mc[lh*128+p, k] and
    st [P, 2*F] with st[p, (pr*K+k)*LH+lh] = +-Ms[lh*128+p, k]
    (+ at pr=0, - at pr=1)."""
    kk = np.arange(0, D, 2, dtype=np.float64) / D
    base = 1.0 / (10000.0**kk)
    t = np.arange(L, dtype=np.float64)
    fr = t[:, None] * base[None, :]
    fcos, fsin = np.cos(fr), np.sin(fr)
    f0 = 1.0 + float(gate_val) * (0.0 - 0.5) * 0.1
    Mc = np.empty((L, K))
    Ms = np.empty((L, K))
    Mc[1:] = (1 - f0) * fcos[:-1] + f0 * fcos[1:]
    Ms[1:] = (1 - f0) * fsin[:-1] + f0 * fsin[1:]
    Mc[0], Ms[0] = 1.0, 0.0
    # [L, K] -> [LH, P, K] -> [P, K, LH]
    Mc = Mc.reshape(LH, P, K).transpose(1, 2, 0)
    Ms = Ms.reshape(LH, P, K).transpose(1, 2, 0)
    ct = np.ascontiguousarray(Mc).astype(dt_np).reshape(P, F)
    st = np.stack([Ms, -Ms], axis=1)  # [P, 2, K, LH]
    st = np.ascontiguousarray(st).astype(dt_np).reshape(P, 2 * F)
    return ct, st


def _pack(x, gate_val, dt_np):
    """Host prep: per-core x [B, P, H*2*F] (layout (h, pr, k, lh) per
    partition) + tables."""
    ct, st = _tables(gate_val, dt_np)
    # x [B, L, H, D]; l = lh*P + p, d = k*2 + pr
    xr = x.astype(dt_np).reshape(B, LH, P, H, K, 2)
    xd = np.ascontiguousarray(xr.transpose(0, 2, 3, 5, 4, 1)).reshape(B, P, H * 2 * F)
    return xd, ct, st


def _inmaps(x, gate_val, dt_np):
    xd, ct, st = _pack(x, gate_val, dt_np)
    use_te = "T" in ADDMIX
    iden = np.eye(P, dtype=dt_np) if use_te else None
    maps = []
    for i in range(NCORES):
        m = {"x": xd[i], "ct": ct, "st": st}
        if use_te:
            m["iden"] = iden
        maps.append(m)
    return maps


def _unpack(outs, dtype):
    # outs [B, P, H*2*F] -> [B, LH, P, H, K, pr] -> [B, L, H, D]
    o = outs.reshape(B, P, H, 2, K, LH).transpose(0, 5, 1, 2, 4, 3)
    return np.ascontiguousarray(o).reshape(B, L, H, D).astype(dtype)


def kernel(x, W, b, gate):
    dt_np = np.float16 if F16 else np.float32
    x = np.asarray(x)
    gate_val = np.asarray(gate).reshape(-1)[0]

    key = dt_np
    if key not in _cache:
        _cache[key] = _build(dt_np)
    nc = _cache[key]

    in_maps = _inmaps(x, gate_val, dt_np)
    res = run_bass_kernel_spmd(nc, in_maps, list(range(NCORES)))
    outs = np.stack([res.results[i]["out"] for i in range(NCORES)])
    return _unpack(outs, x.dtype)


# revision 3
# speedup vs baseline: 1.1534x; 1.1534x over previous
"""Trainium2 kernel for nn_BetweennessRoPE.

Mathematical background
-----------------------
The reference computes a "betweenness"-adjusted interpolated RoPE:

    adjust      = gate * (betweenness - 0.5) * 0.1
    adj_pos     = clip(pos + adjust, 0, 2047)
    cos_i/sin_i = lerp of the cos/sin tables at floor/ceil(adj_pos)
    out         = rotate(x, cos_i, sin_i)

By the triangle inequality path >= direct, so score in [0, 1] and
betweenness in [0, 1/(L-2)].  Hence

    adjust = gate*0.05*betweenness - gate*0.05  in  (-0.025, -0.0249756]

is always a small negative number: floor/ceil(pos + adjust) = (pos-1, pos)
for every pos >= 1 (and pos 0 clips to exactly 0).  The interpolation
therefore uses *statically known* table rows, with fraction

    frac = 1 + adjust = f0 + eps,   f0 = 1 - 0.05*gate,
    eps  = gate*0.05*betweenness  in  [0, gate*0.05/(L-2)]  (~2.4e-5)

The eps-dependent part of the output is bounded by |eps * x|, two orders
of magnitude below the test gate, so the kernel applies the lerped
rotation at fixed fraction f0 with host-precomputed tables

    Mc[l] = (1-f0)*cos((l-1)*theta) + f0*cos(l*theta)   (l >= 1)
    Ms[l] = (1-f0)*sin((l-1)*theta) + f0*sin(l*theta)
    Mc[0] = 1, Ms[0] = 0                                (pos-0 clips to 0)

and the device kernel is a pure broadcast complex-multiply:

    out_even = x_even*Mc - x_odd*Ms
    out_odd  = x_odd *Mc + x_even*Ms

Memory-bound; data-parallel over batch (core i handles batch i).

Device layout (per core)
------------------------
x is sent de-interleaved in fp16 with per-partition free layout
(h, pr, k, lh) and partitions p = l % 128 (l = lh*128 + p):

    xd[p, h, pr, k, lh]     flat [128, 16384]

so every DMA slice along h is fully contiguous per partition, and the
cos/sin table operands (which do not depend on h) are contiguous
512-element runs broadcast with a stride-0 h (and pr, for cos) axis —
the cheapest possible AP for the DVE's fp16 2x mode.

Per head-group: tP = x*C and tQ = x*(+-S) on DVE (2x fp16), then
out = tP + parity-swap(tQ) on a per-group engine: DVE for the first
and last group (fast ramp / short tail), TensorE identity-matmul
accumulation + ScalarE PSUM->SBUF cast for middle groups, GPSIMD
tensor_tensor for the rest — balancing all engines under the ~23 us
HBM roofline for the 8 MiB of fp16 traffic.
"""

import os
import sys

import numpy as np

for _p in ("/opt/trn_rl_repo",):
    if _p not in sys.path and os.path.isdir(_p):
        sys.path.insert(0, _p)

import concourse.tile as tile  # noqa: E402
from concourse import bacc, mybir  # noqa: E402
from concourse.bass_utils import run_bass_kernel_spmd  # noqa: E402

B, L, H, D = 8, 2048, 16, 64
K = D // 2  # 32
P = 128  # partitions
LH = L // P  # 16 l_hi values
F = K * LH  # 512: per-(h,pr) contiguous run length
NCORES = 8

# Tunables
# h-split per pipeline group (must sum to H=16) and per-group add engine:
# V = DVE tensor_tensor, T = TensorE identity-matmul + ScalarE cast,
# G = GPSIMD tensor_tensor.
HSPLIT = [int(s) for s in os.environ.get("ROPE_HSPLIT", "1,2,2,2,2,2,2,2,1").split(",")]
ADDMIX = os.environ.get("ROPE_ADDMIX", "VTTTTVVVV")
F16 = os.environ.get("ROPE_F16", "1") == "1"  # fp16 pipeline (else fp32)

_cache = {}


def _build(dt_np):
    """Build the Bass program (shared by all 8 cores)."""
    assert sum(HSPLIT) == H and len(ADDMIX) == len(HSPLIT)
    use_te = "T" in ADDMIX
    dt = mybir.dt.float16 if dt_np == np.float16 else mybir.dt.float32
    nc = bacc.Bacc(
        "TRN2",
        target_bir_lowering=False,
        debug=False,
        enable_asserts=False,
        num_devices=NCORES,
    )
    xin = nc.dram_tensor("x", [P, H * 2 * F], dt, kind="ExternalInput")
    ctd = nc.dram_tensor("ct", [P, F], dt, kind="ExternalInput")
    std = nc.dram_tensor("st", [P, 2 * F], dt, kind="ExternalInput")
    if use_te:
        idd = nc.dram_tensor("iden", [P, P], dt, kind="ExternalInput")
    out = nc.dram_tensor("out", [P, H * 2 * F], dt, kind="ExternalOutput")

    from contextlib import ExitStack

    with tile.TileContext(nc) as tc, ExitStack() as ctx:
        tabp = ctx.enter_context(tc.tile_pool(name="tab", bufs=1))
        xp = ctx.enter_context(tc.tile_pool(name="xin", bufs=6))
        op_ = ctx.enter_context(tc.tile_pool(name="out", bufs=4))
        tp = ctx.enter_context(tc.tile_pool(name="tmp", bufs=4))
        olp = ctx.enter_context(tc.tile_pool(name="outl", bufs=1))
        if use_te:
            psp = ctx.enter_context(tc.tile_pool(name="ps", bufs=2, space="PSUM"))

        mult = mybir.AluOpType.mult
        add = mybir.AluOpType.add

        # first x load issues before everything else on the sync ring;
        # tables + identity go down the scalar ring in parallel
        h0 = 0
        nh0 = HSPLIT[0]
        xt0 = xp.tile([P, nh0 * 2 * F], dt, tag="xt")
        nc.sync.dma_start(xt0[:], xin[:, : nh0 * 2 * F])
        ctt = tabp.tile([P, F], dt)
        nc.scalar.dma_start(ctt[:], ctd[:])
        stt = tabp.tile([P, 2 * F], dt)
        nc.scalar.dma_start(stt[:], std[:])
        if use_te:
            idt = tabp.tile([P, P], dt)
            nc.scalar.dma_start(idt[:], idd[:])

        nstore_scalar = (len(HSPLIT) + 1) // 2
        for g, (nh, eng) in enumerate(zip(HSPLIT, ADDMIX)):
            gf = nh * 2 * F
            if g == 0:
                xt = xt0
            else:
                xt = xp.tile([P, gf], dt, tag="xt")
                nc.sync.dma_start(xt[:], xin[:, h0 * 2 * F : h0 * 2 * F + gf])

            # dedicated tile for the last group's output: its combine must
            # never wait on a store-slot recycle
            if g == len(HSPLIT) - 1:
                ot = olp.tile([P, gf], dt)
            else:
                ot = op_.tile([P, gf], dt, tag="ot")

            xv = xt[:].rearrange("p (h pr f) -> p h pr f", h=nh, pr=2)
            ov = ot[:].rearrange("p (h pr f) -> p h pr f", h=nh, pr=2)
            # cos: broadcast over (h, pr); sin: sign-folded per parity,
            # broadcast over h only.  Both have contiguous 512-elem runs.
            C = ctt[:].unsqueeze(1).unsqueeze(1).broadcast_to([P, nh, 2, F])
            S2 = stt[:].rearrange("p (pr f) -> p pr f", pr=2).unsqueeze(1)
            S2 = S2.broadcast_to([P, nh, 2, F])

            tP = tp.tile([P, gf], dt, tag="tP")
            tQ = tp.tile([P, gf], dt, tag="tQ")
            tPv = tP[:].rearrange("p (h pr f) -> p h pr f", h=nh, pr=2)
            tQv = tQ[:].rearrange("p (h pr f) -> p h pr f", h=nh, pr=2)

            # tP = x*C ; tQ = x*(+-S) ; out = tP + parity-swap(tQ):
            #   out_even = E*C + (O*-S) ; out_odd = O*C + (E*+S)
            nc.vector.tensor_tensor(tPv, xv, C, mult)
            nc.vector.tensor_tensor(tQv, xv, S2, mult)

            if eng == "T":
                # the add runs on TensorE as identity-matmul accumulation
                # into PSUM; ScalarE casts PSUM f32 -> SBUF fp16.  A
                # 512-col chunk is one (h, pr) slot; its parity partner
                # is chunk c^1.
                ps = psp.tile([P, gf], mybir.dt.float32, tag="ps")
                for c in range(gf // 512):
                    pch = tP[:, c * 512 : (c + 1) * 512]
                    qch = tQ[:, (c ^ 1) * 512 : ((c ^ 1) + 1) * 512]
                    po = ps[:, c * 512 : (c + 1) * 512]
                    nc.tensor.matmul(po, idt[:], pch, start=True, stop=False)
                    nc.tensor.matmul(po, idt[:], qch, start=False, stop=True)
                nc.scalar.copy(ot[:], ps[:])
            elif eng == "G":
                nc.gpsimd.tensor_tensor(ov, tPv, tQv[:, :, ::-1, :], add)
            else:
                nc.vector.tensor_tensor(ov, tPv, tQv[:, :, ::-1, :], add)

            # stores: first half on the scalar ring, second half on sync
            # (whose loads are all queued by then) to split trigger cost
            if g < nstore_scalar:
                nc.scalar.dma_start(out[:, h0 * 2 * F : h0 * 2 * F + gf], ot[:])
            else:
                nc.sync.dma_start(out[:, h0 * 2 * F : h0 * 2 * F + gf], ot[:])
            h0 += nh

    nc.compile()
    return nc


def _tables(gate_val, dt_np):
    """Host-precomputed lerped cos/sin tables.

    Returns ct [P, F] with ct[p, k*LH+lh] = Mc[lh*128+p, k] and
    st [P, 2*F] with st[p, (pr*K+k)*LH+lh] = +-Ms[lh*128+p, k]
    (+ at pr=0, - at pr=1)."""
    kk = np.arange(0, D, 2, dtype=np.float64) / D
    base = 1.0 / (10000.0**kk)
    t = np.arange(L, dtype=np.float64)
    fr = t[:, None] * base[None, :]
    fcos, fsin = np.cos(fr), np.sin(fr)
    f0 = 1.0 + float(gate_val) * (0.0 - 0.5) * 0.1
    Mc = np.empty((L, K))
    Ms = np.empty((L, K))
    Mc[1:] = (1 - f0) * fcos[:-1] + f0 * fcos[1:]
    Ms[1:] = (1 - f0) * fsin[:-1] + f0 * fsin[1:]
    Mc[0], Ms[0] = 1.0, 0.0
    # [L, K] -> [LH, P, K] -> [P, K, LH]
    Mc = Mc.reshape(LH, P, K).transpose(1, 2, 0)
    Ms = Ms.reshape(LH, P, K).transpose(1, 2, 0)
    ct = np.ascontiguousarray(Mc).astype(dt_np).reshape(P, F)
    st = np.stack([Ms, -Ms], axis=1)  # [P, 2, K, LH]
    st = np.ascontiguousarray(st).astype(dt_np).reshape(P, 2 * F)
    return ct, st


def _pack(x, gate_val, dt_np):
    """Host prep: per-core x [B, P, H*2*F] (layout (h, pr, k, lh) per
    partition) + tables."""
    ct, st = _tables(gate_val, dt_np)
    # x [B, L, H, D]; l = lh*P + p, d = k*2 + pr
    xr = x.astype(dt_np).reshape(B, LH, P, H, K, 2)
    xd = np.ascontiguousarray(xr.transpose(0, 2, 3, 5, 4, 1)).reshape(B, P, H * 2 * F)
    return xd, ct, st


def _inmaps(x, gate_val, dt_np):
    xd, ct, st = _pack(x, gate_val, dt_np)
    use_te = "T" in ADDMIX
    iden = np.eye(P, dtype=dt_np) if use_te else None
    maps = []
    for i in range(NCORES):
        m = {"x": xd[i], "ct": ct, "st": st}
        if use_te:
            m["iden"] = iden
        maps.append(m)
    return maps


def _unpack(outs, dtype):
    # outs [B, P, H*2*F] -> [B, LH, P, H, K, pr] -> [B, L, H, D]
    o = outs.reshape(B, P, H, 2, K, LH).transpose(0, 5, 1, 2, 4, 3)
    return np.ascontiguousarray(o).reshape(B, L, H, D).astype(dtype)


def kernel(x, W, b, gate):
    dt_np = np.float16 if F16 else np.float32
    x = np.asarray(x)
    gate_val = np.asarray(gate).reshape(-1)[0]

    key = dt_np
    if key not in _cache:
        _cache[key] = _build(dt_np)
    nc = _cache[key]

    in_maps = _inmaps(x, gate_val, dt_np)
    res = run_bass_kernel_spmd(nc, in_maps, list(range(NCORES)))
    outs = np.stack([res.results[i]["out"] for i in range(NCORES)])
    return _unpack(outs, x.dtype)


# revision 4
# speedup vs baseline: 1.1802x; 1.0232x over previous
"""Trainium2 kernel for nn_BetweennessRoPE.

Mathematical background
-----------------------
The reference computes a "betweenness"-adjusted interpolated RoPE:

    adjust      = gate * (betweenness - 0.5) * 0.1
    adj_pos     = clip(pos + adjust, 0, 2047)
    cos_i/sin_i = lerp of the cos/sin tables at floor/ceil(adj_pos)
    out         = rotate(x, cos_i, sin_i)

By the triangle inequality path >= direct, so score in [0, 1] and
betweenness in [0, 1/(L-2)].  Hence

    adjust = gate*0.05*betweenness - gate*0.05  in  (-0.025, -0.0249756]

is always a small negative number: floor/ceil(pos + adjust) = (pos-1, pos)
for every pos >= 1 (and pos 0 clips to exactly 0).  The interpolation
therefore uses *statically known* table rows, with fraction

    frac = 1 + adjust = f0 + eps,   f0 = 1 - 0.05*gate,
    eps  = gate*0.05*betweenness  in  [0, gate*0.05/(L-2)]  (~2.4e-5)

The eps-dependent part of the output is bounded by |eps * x|, two orders
of magnitude below the test gate, so the kernel applies the lerped
rotation at fixed fraction f0 with host-precomputed tables

    Mc[l] = (1-f0)*cos((l-1)*theta) + f0*cos(l*theta)   (l >= 1)
    Ms[l] = (1-f0)*sin((l-1)*theta) + f0*sin(l*theta)
    Mc[0] = 1, Ms[0] = 0                                (pos-0 clips to 0)

and the device kernel is a pure broadcast complex-multiply:

    out_even = x_even*Mc - x_odd*Ms
    out_odd  = x_odd *Mc + x_even*Ms

Memory-bound; data-parallel over batch (core i handles batch i).

Device layout (per core)
------------------------
x is sent de-interleaved in fp16 with per-partition free layout
(h, pr, k, lh) and partitions p = l % 128 (l = lh*128 + p):

    xd[p, h, pr, k, lh]     flat [128, 16384]

so every DMA slice along h is fully contiguous per partition, and the
cos/sin table operands (which do not depend on h) are contiguous
512-element runs broadcast with a stride-0 h (and pr, for cos) axis —
the cheapest possible AP for the DVE's fp16 2x mode.

Per head-group: tP = x*C and tQ = x*(+-S) on DVE (2x fp16), then
out = tP + parity-swap(tQ) on a per-group engine: DVE for the first
and last group (fast ramp / short tail), TensorE identity-matmul
accumulation + ScalarE PSUM->SBUF cast for middle groups, GPSIMD
tensor_tensor for the rest — balancing all engines under the ~23 us
HBM roofline for the 8 MiB of fp16 traffic.
"""

import os
import sys

import numpy as np

for _p in ("/opt/trn_rl_repo",):
    if _p not in sys.path and os.path.isdir(_p):
        sys.path.insert(0, _p)

import concourse.tile as tile  # noqa: E402
from concourse import bacc, mybir  # noqa: E402
from concourse.bass_utils import run_bass_kernel_spmd  # noqa: E402

B, L, H, D = 8, 2048, 16, 64
K = D // 2  # 32
P = 128  # partitions
LH = L // P  # 16 l_hi values
F = K * LH  # 512: per-(h,pr) contiguous run length
NCORES = 8

# Tunables
# h-split per pipeline group (must sum to H=16) and per-group add engine:
# V = DVE tensor_tensor, T = TensorE identity-matmul + ScalarE cast,
# G = GPSIMD tensor_tensor.
HSPLIT = [int(s) for s in os.environ.get("ROPE_HSPLIT", "1,2,2,2,2,2,2,2,1").split(",")]
ADDMIX = os.environ.get("ROPE_ADDMIX", "VTTTTVVVV")
F16 = os.environ.get("ROPE_F16", "1") == "1"  # fp16 pipeline (else fp32)

_cache = {}


def _build(dt_np):
    """Build the Bass program (shared by all 8 cores)."""
    assert sum(HSPLIT) == H and len(ADDMIX) == len(HSPLIT)
    use_te = "T" in ADDMIX
    dt = mybir.dt.float16 if dt_np == np.float16 else mybir.dt.float32
    nc = bacc.Bacc(
        "TRN2",
        target_bir_lowering=False,
        debug=False,
        enable_asserts=False,
        num_devices=NCORES,
    )
    xin = nc.dram_tensor("x", [P, H * 2 * F], dt, kind="ExternalInput")
    tbd = nc.dram_tensor("tab", [P, 3 * F], dt, kind="ExternalInput")
    if use_te:
        idd = nc.dram_tensor("iden", [P, P], dt, kind="ExternalInput")
    out = nc.dram_tensor("out", [P, H * 2 * F], dt, kind="ExternalOutput")

    from contextlib import ExitStack

    with tile.TileContext(nc) as tc, ExitStack() as ctx:
        tabp = ctx.enter_context(tc.tile_pool(name="tab", bufs=1))
        xp = ctx.enter_context(tc.tile_pool(name="xin", bufs=9))
        op_ = ctx.enter_context(tc.tile_pool(name="out", bufs=8))
        tp = ctx.enter_context(tc.tile_pool(name="tmp", bufs=6))
        olp = ctx.enter_context(tc.tile_pool(name="outl", bufs=1))
        if use_te:
            psp = ctx.enter_context(tc.tile_pool(name="ps", bufs=2, space="PSUM"))

        mult = mybir.AluOpType.mult
        add = mybir.AluOpType.add

        # tables first on the sync ring (they gate the very first mult),
        # then the x loads; identity goes down the scalar ring in parallel
        tbt = tabp.tile([P, 3 * F], dt)
        nc.sync.dma_start(tbt[:], tbd[:])
        ctt = tbt[:, :F]
        stt = tbt[:, F:]
        if use_te:
            idt = tabp.tile([P, P], dt)
            nc.scalar.dma_start(idt[:], idd[:])
        h0 = 0
        nh0 = HSPLIT[0]
        xt0 = xp.tile([P, nh0 * 2 * F], dt, tag="xt")
        nc.sync.dma_start(xt0[:], xin[:, : nh0 * 2 * F])

        nstore_scalar = (len(HSPLIT) + 1) // 2
        for g, (nh, eng) in enumerate(zip(HSPLIT, ADDMIX)):
            gf = nh * 2 * F
            if g == 0:
                xt = xt0
            else:
                xt = xp.tile([P, gf], dt, tag="xt")
                nc.sync.dma_start(xt[:], xin[:, h0 * 2 * F : h0 * 2 * F + gf])

            # dedicated tile for the last group's output: its combine must
            # never wait on a store-slot recycle
            if g == len(HSPLIT) - 1:
                ot = olp.tile([P, gf], dt)
            else:
                ot = op_.tile([P, gf], dt, tag="ot")

            xv = xt[:].rearrange("p (h pr f) -> p h pr f", h=nh, pr=2)
            ov = ot[:].rearrange("p (h pr f) -> p h pr f", h=nh, pr=2)
            # cos: broadcast over (h, pr); sin: sign-folded per parity,
            # broadcast over h only.  Both have contiguous 512-elem runs.
            C = ctt.unsqueeze(1).unsqueeze(1).broadcast_to([P, nh, 2, F])
            S2 = stt.rearrange("p (pr f) -> p pr f", pr=2).unsqueeze(1)
            S2 = S2.broadcast_to([P, nh, 2, F])

            tP = tp.tile([P, gf], dt, tag="tP")
            tQ = tp.tile([P, gf], dt, tag="tQ")
            tPv = tP[:].rearrange("p (h pr f) -> p h pr f", h=nh, pr=2)
            tQv = tQ[:].rearrange("p (h pr f) -> p h pr f", h=nh, pr=2)

            # tP = x*C ; tQ = x*(+-S) ; out = tP + parity-swap(tQ):
            #   out_even = E*C + (O*-S) ; out_odd = O*C + (E*+S)
            nc.vector.tensor_tensor(tPv, xv, C, mult)
            nc.vector.tensor_tensor(tQv, xv, S2, mult)

            if eng == "T":
                # the add runs on TensorE as identity-matmul accumulation
                # into PSUM; ScalarE casts PSUM f32 -> SBUF fp16.  A
                # 512-col chunk is one (h, pr) slot; its parity partner
                # is chunk c^1.
                ps = psp.tile([P, gf], mybir.dt.float32, tag="ps")
                for c in range(gf // 512):
                    pch = tP[:, c * 512 : (c + 1) * 512]
                    qch = tQ[:, (c ^ 1) * 512 : ((c ^ 1) + 1) * 512]
                    po = ps[:, c * 512 : (c + 1) * 512]
                    nc.tensor.matmul(po, idt[:], pch, start=True, stop=False)
                    nc.tensor.matmul(po, idt[:], qch, start=False, stop=True)
                nc.scalar.copy(ot[:], ps[:])
            elif eng == "G":
                nc.gpsimd.tensor_tensor(ov, tPv, tQv[:, :, ::-1, :], add)
            else:
                nc.vector.tensor_tensor(ov, tPv, tQv[:, :, ::-1, :], add)

            # stores: first half on the scalar ring, second half on sync
            # (whose loads are all queued by then) to split trigger cost
            if g < nstore_scalar or g == len(HSPLIT) - 1:
                nc.scalar.dma_start(out[:, h0 * 2 * F : h0 * 2 * F + gf], ot[:])
            else:
                nc.sync.dma_start(out[:, h0 * 2 * F : h0 * 2 * F + gf], ot[:])
            h0 += nh

    nc.compile()
    return nc


def _tables(gate_val, dt_np):
    """Host-precomputed lerped cos/sin tables.

    Returns ct [P, F] with ct[p, k*LH+lh] = Mc[lh*128+p, k] and
    st [P, 2*F] with st[p, (pr*K+k)*LH+lh] = +-Ms[lh*128+p, k]
    (+ at pr=0, - at pr=1)."""
    kk = np.arange(0, D, 2, dtype=np.float64) / D
    base = 1.0 / (10000.0**kk)
    t = np.arange(L, dtype=np.float64)
    fr = t[:, None] * base[None, :]
    fcos, fsin = np.cos(fr), np.sin(fr)
    f0 = 1.0 + float(gate_val) * (0.0 - 0.5) * 0.1
    Mc = np.empty((L, K))
    Ms = np.empty((L, K))
    Mc[1:] = (1 - f0) * fcos[:-1] + f0 * fcos[1:]
    Ms[1:] = (1 - f0) * fsin[:-1] + f0 * fsin[1:]
    Mc[0], Ms[0] = 1.0, 0.0
    # [L, K] -> [LH, P, K] -> [P, K, LH]
    Mc = Mc.reshape(LH, P, K).transpose(1, 2, 0)
    Ms = Ms.reshape(LH, P, K).transpose(1, 2, 0)
    ct = np.ascontiguousarray(Mc).astype(dt_np).reshape(P, F)
    st = np.stack([Ms, -Ms], axis=1)  # [P, 2, K, LH]
    st = np.ascontiguousarray(st).astype(dt_np).reshape(P, 2 * F)
    return ct, st


def _pack(x, gate_val, dt_np):
    """Host prep: per-core x [B, P, H*2*F] (layout (h, pr, k, lh) per
    partition) + tables."""
    ct, st = _tables(gate_val, dt_np)
    # x [B, L, H, D]; l = lh*P + p, d = k*2 + pr
    xr = x.astype(dt_np).reshape(B, LH, P, H, K, 2)
    xd = np.ascontiguousarray(xr.transpose(0, 2, 3, 5, 4, 1)).reshape(B, P, H * 2 * F)
    return xd, ct, st


def _inmaps(x, gate_val, dt_np):
    xd, ct, st = _pack(x, gate_val, dt_np)
    tab = np.concatenate([ct, st], axis=1)
    use_te = "T" in ADDMIX
    iden = np.eye(P, dtype=dt_np) if use_te else None
    maps = []
    for i in range(NCORES):
        m = {"x": xd[i], "tab": tab}
        if use_te:
            m["iden"] = iden
        maps.append(m)
    return maps


def _unpack(outs, dtype):
    # outs [B, P, H*2*F] -> [B, LH, P, H, K, pr] -> [B, L, H, D]
    o = outs.reshape(B, P, H, 2, K, LH).transpose(0, 5, 1, 2, 4, 3)
    return np.ascontiguousarray(o).reshape(B, L, H, D).astype(dtype)


def kernel(x, W, b, gate):
    dt_np = np.float16 if F16 else np.float32
    x = np.asarray(x)
    gate_val = np.asarray(gate).reshape(-1)[0]

    key = dt_np
    if key not in _cache:
        _cache[key] = _build(dt_np)
    nc = _cache[key]

    in_maps = _inmaps(x, gate_val, dt_np)
    res = run_bass_kernel_spmd(nc, in_maps, list(range(NCORES)))
    outs = np.stack([res.results[i]["out"] for i in range(NCORES)])
    return _unpack(outs, x.dtype)


# revision 5
# speedup vs baseline: 1.1836x; 1.0029x over previous
"""Trainium2 kernel for nn_BetweennessRoPE.

Mathematical background
-----------------------
The reference computes a "betweenness"-adjusted interpolated RoPE:

    adjust      = gate * (betweenness - 0.5) * 0.1
    adj_pos     = clip(pos + adjust, 0, 2047)
    cos_i/sin_i = lerp of the cos/sin tables at floor/ceil(adj_pos)
    out         = rotate(x, cos_i, sin_i)

By the triangle inequality path >= direct, so score in [0, 1] and
betweenness in [0, 1/(L-2)].  Hence

    adjust = gate*0.05*betweenness - gate*0.05  in  (-0.025, -0.0249756]

is always a small negative number: floor/ceil(pos + adjust) = (pos-1, pos)
for every pos >= 1 (and pos 0 clips to exactly 0).  The interpolation
therefore uses *statically known* table rows, with fraction

    frac = 1 + adjust = f0 + eps,   f0 = 1 - 0.05*gate,
    eps  = gate*0.05*betweenness  in  [0, gate*0.05/(L-2)]  (~2.4e-5)

The eps-dependent part of the output is bounded by |eps * x|, two orders
of magnitude below the test gate, so the kernel applies the lerped
rotation at fixed fraction f0 with host-precomputed tables

    Mc[l] = (1-f0)*cos((l-1)*theta) + f0*cos(l*theta)   (l >= 1)
    Ms[l] = (1-f0)*sin((l-1)*theta) + f0*sin(l*theta)
    Mc[0] = 1, Ms[0] = 0                                (pos-0 clips to 0)

and the device kernel is a pure broadcast complex-multiply:

    out_even = x_even*Mc - x_odd*Ms
    out_odd  = x_odd *Mc + x_even*Ms

Memory-bound; data-parallel over batch (core i handles batch i).

Device layout (per core)
------------------------
x is sent de-interleaved in fp16 with per-partition free layout
(h, pr, k, lh) and partitions p = l % 128 (l = lh*128 + p):

    xd[p, h, pr, k, lh]     flat [128, 16384]

so every DMA slice along h is fully contiguous per partition, and the
cos/sin table operands (which do not depend on h) are contiguous
512-element runs broadcast with a stride-0 h (and pr, for cos) axis —
the cheapest possible AP for the DVE's fp16 2x mode.

Per head-group: tP = x*C and tQ = x*(+-S) on DVE (2x fp16), then
out = tP + parity-swap(tQ) on a per-group engine: DVE for the first
and last group (fast ramp / short tail), TensorE identity-matmul
accumulation + ScalarE PSUM->SBUF cast for middle groups, GPSIMD
tensor_tensor for the rest — balancing all engines under the ~23 us
HBM roofline for the 8 MiB of fp16 traffic.
"""

import os
import sys

import numpy as np

for _p in ("/opt/trn_rl_repo",):
    if _p not in sys.path and os.path.isdir(_p):
        sys.path.insert(0, _p)

import concourse.tile as tile  # noqa: E402
from concourse import bacc, mybir  # noqa: E402
from concourse.bass_utils import run_bass_kernel_spmd  # noqa: E402

B, L, H, D = 8, 2048, 16, 64
K = D // 2  # 32
P = 128  # partitions
LH = L // P  # 16 l_hi values
F = K * LH  # 512: per-(h,pr) contiguous run length
NCORES = 8

# Tunables
# h-split per pipeline group (must sum to H=16) and per-group add engine:
# V = DVE tensor_tensor, T = TensorE identity-matmul + ScalarE cast,
# G = GPSIMD tensor_tensor.
HSPLIT = [int(s) for s in os.environ.get("ROPE_HSPLIT", "1,2,2,2,2,2,2,2,1").split(",")]
ADDMIX = os.environ.get("ROPE_ADDMIX", "VTTTTTVVV")
F16 = os.environ.get("ROPE_F16", "1") == "1"  # fp16 pipeline (else fp32)

_cache = {}


def _build(dt_np):
    """Build the Bass program (shared by all 8 cores)."""
    assert sum(HSPLIT) == H and len(ADDMIX) == len(HSPLIT)
    use_te = "T" in ADDMIX
    dt = mybir.dt.float16 if dt_np == np.float16 else mybir.dt.float32
    nc = bacc.Bacc(
        "TRN2",
        target_bir_lowering=False,
        debug=False,
        enable_asserts=False,
        num_devices=NCORES,
    )
    xin = nc.dram_tensor("x", [P, H * 2 * F], dt, kind="ExternalInput")
    ctd = nc.dram_tensor("ct", [P, F], dt, kind="ExternalInput")
    std = nc.dram_tensor("st", [P, 2 * F], dt, kind="ExternalInput")
    if use_te:
        idd = nc.dram_tensor("iden", [P, P], dt, kind="ExternalInput")
    out = nc.dram_tensor("out", [P, H * 2 * F], dt, kind="ExternalOutput")

    from contextlib import ExitStack

    with tile.TileContext(nc) as tc, ExitStack() as ctx:
        tabp = ctx.enter_context(tc.tile_pool(name="tab", bufs=1))
        xp = ctx.enter_context(tc.tile_pool(name="xin", bufs=9))
        op_ = ctx.enter_context(tc.tile_pool(name="out", bufs=8))
        tp = ctx.enter_context(tc.tile_pool(name="tmp", bufs=6))
        olp = ctx.enter_context(tc.tile_pool(name="outl", bufs=1))
        if use_te:
            psp = ctx.enter_context(tc.tile_pool(name="ps", bufs=2, space="PSUM"))

        mult = mybir.AluOpType.mult
        add = mybir.AluOpType.add

        # table loads bracket the first x load on the sync ring: the tiny
        # cos table (gating the very first mult) goes first, the sin table
        # right after xt0; identity goes down the scalar ring in parallel
        ctt_t = tabp.tile([P, F], dt)
        nc.sync.dma_start(ctt_t[:], ctd[:])
        h0 = 0
        nh0 = HSPLIT[0]
        xt0 = xp.tile([P, nh0 * 2 * F], dt, tag="xt")
        nc.sync.dma_start(xt0[:], xin[:, : nh0 * 2 * F])
        stt_t = tabp.tile([P, 2 * F], dt)
        nc.sync.dma_start(stt_t[:], std[:])
        ctt = ctt_t[:]
        stt = stt_t[:]
        if use_te:
            idt = tabp.tile([P, P], dt)
            nc.scalar.dma_start(idt[:], idd[:])

        nstore_scalar = (len(HSPLIT) + 1) // 2
        for g, (nh, eng) in enumerate(zip(HSPLIT, ADDMIX)):
            gf = nh * 2 * F
            if g == 0:
                xt = xt0
            else:
                xt = xp.tile([P, gf], dt, tag="xt")
                nc.sync.dma_start(xt[:], xin[:, h0 * 2 * F : h0 * 2 * F + gf])

            # dedicated tile for the last group's output: its combine must
            # never wait on a store-slot recycle
            if g == len(HSPLIT) - 1:
                ot = olp.tile([P, gf], dt)
            else:
                ot = op_.tile([P, gf], dt, tag="ot")

            xv = xt[:].rearrange("p (h pr f) -> p h pr f", h=nh, pr=2)
            ov = ot[:].rearrange("p (h pr f) -> p h pr f", h=nh, pr=2)
            # cos: broadcast over (h, pr); sin: sign-folded per parity,
            # broadcast over h only.  Both have contiguous 512-elem runs.
            C = ctt.unsqueeze(1).unsqueeze(1).broadcast_to([P, nh, 2, F])
            S2 = stt.rearrange("p (pr f) -> p pr f", pr=2).unsqueeze(1)
            S2 = S2.broadcast_to([P, nh, 2, F])

            tP = tp.tile([P, gf], dt, tag="tP")
            tQ = tp.tile([P, gf], dt, tag="tQ")
            tPv = tP[:].rearrange("p (h pr f) -> p h pr f", h=nh, pr=2)
            tQv = tQ[:].rearrange("p (h pr f) -> p h pr f", h=nh, pr=2)

            # tP = x*C ; tQ = x*(+-S) ; out = tP + parity-swap(tQ):
            #   out_even = E*C + (O*-S) ; out_odd = O*C + (E*+S)
            nc.vector.tensor_tensor(tPv, xv, C, mult)
            nc.vector.tensor_tensor(tQv, xv, S2, mult)

            if eng == "T":
                # the add runs on TensorE as identity-matmul accumulation
                # into PSUM; ScalarE casts PSUM f32 -> SBUF fp16.  A
                # 512-col chunk is one (h, pr) slot; its parity partner
                # is chunk c^1.
                ps = psp.tile([P, gf], mybir.dt.float32, tag="ps")
                for c in range(gf // 512):
                    pch = tP[:, c * 512 : (c + 1) * 512]
                    qch = tQ[:, (c ^ 1) * 512 : ((c ^ 1) + 1) * 512]
                    po = ps[:, c * 512 : (c + 1) * 512]
                    nc.tensor.matmul(po, idt[:], pch, start=True, stop=False)
                    nc.tensor.matmul(po, idt[:], qch, start=False, stop=True)
                nc.scalar.copy(ot[:], ps[:])
            elif eng == "G":
                nc.gpsimd.tensor_tensor(ov, tPv, tQv[:, :, ::-1, :], add)
            else:
                nc.vector.tensor_tensor(ov, tPv, tQv[:, :, ::-1, :], add)

            # stores: first half on the scalar ring, second half on sync
            # (whose loads are all queued by then) to split trigger cost
            if g < nstore_scalar or g == len(HSPLIT) - 1:
                nc.scalar.dma_start(out[:, h0 * 2 * F : h0 * 2 * F + gf], ot[:])
            else:
                nc.sync.dma_start(out[:, h0 * 2 * F : h0 * 2 * F + gf], ot[:])
            h0 += nh

    nc.compile()
    return nc


def _tables(gate_val, dt_np):
    """Host-precomputed lerped cos/sin tables.

    Returns ct [P, F] with ct[p, k*LH+lh] = Mc[lh*128+p, k] and
    st [P, 2*F] with st[p, (pr*K+k)*LH+lh] = +-Ms[lh*128+p, k]
    (+ at pr=0, - at pr=1)."""
    kk = np.arange(0, D, 2, dtype=np.float64) / D
    base = 1.0 / (10000.0**kk)
    t = np.arange(L, dtype=np.float64)
    fr = t[:, None] * base[None, :]
    fcos, fsin = np.cos(fr), np.sin(fr)
    f0 = 1.0 + float(gate_val) * (0.0 - 0.5) * 0.1
    Mc = np.empty((L, K))
    Ms = np.empty((L, K))
    Mc[1:] = (1 - f0) * fcos[:-1] + f0 * fcos[1:]
    Ms[1:] = (1 - f0) * fsin[:-1] + f0 * fsin[1:]
    Mc[0], Ms[0] = 1.0, 0.0
    # [L, K] -> [LH, P, K] -> [P, K, LH]
    Mc = Mc.reshape(LH, P, K).transpose(1, 2, 0)
    Ms = Ms.reshape(LH, P, K).transpose(1, 2, 0)
    ct = np.ascontiguousarray(Mc).astype(dt_np).reshape(P, F)
    st = np.stack([Ms, -Ms], axis=1)  # [P, 2, K, LH]
    st = np.ascontiguousarray(st).astype(dt_np).reshape(P, 2 * F)
    return ct, st


def _pack(x, gate_val, dt_np):
    """Host prep: per-core x [B, P, H*2*F] (layout (h, pr, k, lh) per
    partition) + tables."""
    ct, st = _tables(gate_val, dt_np)
    # x [B, L, H, D]; l = lh*P + p, d = k*2 + pr
    xr = x.astype(dt_np).reshape(B, LH, P, H, K, 2)
    xd = np.ascontiguousarray(xr.transpose(0, 2, 3, 5, 4, 1)).reshape(B, P, H * 2 * F)
    return xd, ct, st


def _inmaps(x, gate_val, dt_np):
    xd, ct, st = _pack(x, gate_val, dt_np)
    use_te = "T" in ADDMIX
    iden = np.eye(P, dtype=dt_np) if use_te else None
    maps = []
    for i in range(NCORES):
        m = {"x": xd[i], "ct": ct, "st": st}
        if use_te:
            m["iden"] = iden
        maps.append(m)
    return maps


def _unpack(outs, dtype):
    # outs [B, P, H*2*F] -> [B, LH, P, H, K, pr] -> [B, L, H, D]
    o = outs.reshape(B, P, H, 2, K, LH).transpose(0, 5, 1, 2, 4, 3)
    return np.ascontiguousarray(o).reshape(B, L, H, D).astype(dtype)


def kernel(x, W, b, gate):
    dt_np = np.float16 if F16 else np.float32
    x = np.asarray(x)
    gate_val = np.asarray(gate).reshape(-1)[0]

    key = dt_np
    if key not in _cache:
        _cache[key] = _build(dt_np)
    nc = _cache[key]

    in_maps = _inmaps(x, gate_val, dt_np)
    res = run_bass_kernel_spmd(nc, in_maps, list(range(NCORES)))
    outs = np.stack([res.results[i]["out"] for i in range(NCORES)])
    return _unpack(outs, x.dtype)
